# revision 6
# baseline (speedup 1.0000x reference)
"""Trainium2 Bass kernel for nn_BaseLinearSSM (chunked hybrid, fp16).

y[b,t] = Re(C x_{t+1}) + D u[b,t] + bias,  x_{t+1} = A x_t + B u_t  (complex A,B,C)

Strategy (L=8 time chunks, Q=T/L=256 chunks):
  Host (fp64): eigendecompose A = V diag(w) V^-1, fold V into B/C:
  Bt = V^-1 B, Ct = C V.  Chunk the recurrence:

    X_q = w^L X_{q-1} + G_q           (coarse, diagonal complex)
    G_q = sum_s w^(L-1-s) Bt u_{qL+s} (chunk input, a stacked matmul)
    y[qL+j] = Re(Ct diag(w^(j+1)) X_{q-1})          (carry, matmul)
            + sum_{s<=j} P_{j-s} u_{qL+s}            (in-chunk, matmul)
    P_k = Re(Ct diag(w^k) Bt),  P_0 += D

  Device (per core, batch-sharded 2 of 16):
    PE (fp16): G matmuls, carry matmuls, in-chunk triangular matmuls
    DVE: modulate e^{-i.phi.q} -> two real tensor_tensor_scans over the
         Q=256 coarse steps only (8x less scan work than per-step) -> demod
    Act: PSUM->SBUF fp16 copies

  Layout notes: u and y are chunk-major on device (col = b|s|q resp.
  b|j|q, host pre/post-permutes) so every matmul rhs and every output DMA
  is contiguous; both batches share one [128, 2Q] tile per (part, mode
  tile) so each weight LDW feeds two matmuls, and the coarse scans run
  segmented (rho zeroed at the batch-boundary column).
"""

import sys

import numpy as np

if "/opt/trn_rl_repo" not in sys.path:
    sys.path.insert(0, "/opt/trn_rl_repo")

BATCH, T, IN, OUT, N = 16, 2048, 128, 128, 512
NCORES = 8
BLOCAL = BATCH // NCORES  # 2
L = 8                     # time-chunk length
Q = T // L                # 256 coarse steps
Q2 = BLOCAL * Q           # 512 coarse cols, col = b*Q + q
NT = N // 128             # 4 mode tiles
UCOLS = BLOCAL * T        # 4096; u col = (b*L + s)*Q + q  <=>  u[b, q*L+s]

# fp16 blob column layout: u | WG | PW | cos | sin | rho | CW
WG_TILES = 2 * NT * L     # 64, index (n, p, s)
CW_TILES = L * 2 * NT     # 64, index (j, p, n)
OFF_U = 0
OFF_WG = OFF_U + UCOLS
OFF_PW = OFF_WG + WG_TILES * 128
OFF_COS = OFF_PW + L * 128
OFF_SIN = OFF_COS + NT * Q2
OFF_RHO = OFF_SIN + NT * Q2
OFF_CW = OFF_RHO + NT * Q2
BLOBW = OFF_CW + CW_TILES * 128

LAST_RESULT = None
_NC_CACHE = None


def _build_nc():
    from concourse import bass, mybir
    from concourse import tile

    f16 = mybir.dt.bfloat16
    f32 = mybir.dt.float32
    op = mybir.AluOpType

    nc = bass.Bass("TRN2", target_bir_lowering=False, debug=False)

    blob = nc.dram_tensor("blob", [128, BLOBW], f16, kind="ExternalInput")
    # y col = (b*L + j)*Q + q  <=>  y[b, q*L+j]
    yout = nc.dram_tensor("y", [OUT, UCOLS], f16, kind="ExternalOutput")

    with tile.TileContext(nc) as tc:
        with (
            tc.tile_pool(name="const", bufs=1) as cpool,
            tc.tile_pool(name="gsb", bufs=1) as gpool,
            tc.tile_pool(name="tmp", bufs=4) as tpool,
            tc.tile_pool(name="gh", bufs=3) as hpool,
            tc.tile_pool(name="z", bufs=3) as zpool,
            tc.tile_pool(name="xsh", bufs=1) as xpool,
            tc.tile_pool(name="ysb", bufs=1) as ypool,
            tc.tile_pool(name="pg", bufs=4, space="PSUM") as pgpool,
            tc.tile_pool(name="py", bufs=4, space="PSUM") as pypool,
        ):
            blob_sb = cpool.tile([128, BLOBW], f16)
            # DMA pieces in consumption order; two issuing queues.
            WG_HALF = OFF_WG + WG_TILES * 64
            for a, bnd in [(OFF_U, OFF_U + T), (OFF_U + T, OFF_WG),
                           (OFF_WG, WG_HALF), (WG_HALF, OFF_PW)]:
                nc.sync.dma_start(blob_sb[:, a:bnd], blob[:, a:bnd])
            for a, bnd in [(OFF_PW, OFF_CW), (OFF_CW, BLOBW)]:
                nc.scalar.dma_start(blob_sb[:, a:bnd], blob[:, a:bnd])

            def wg(n, p, s):  # G-matmul lhsT tile [128 in, 128 modes]
                i = ((n * 2 + p) * L + s)
                return blob_sb[:, OFF_WG + i * 128:OFF_WG + (i + 1) * 128]

            def pw(k):        # in-chunk lhsT tile [128 in, 128 out]
                return blob_sb[:, OFF_PW + k * 128:OFF_PW + (k + 1) * 128]

            def cw(j, p, n):  # carry lhsT tile [128 modes, 128 out]
                i = (j * 2 + p) * NT + n
                return blob_sb[:, OFF_CW + i * 128:OFF_CW + (i + 1) * 128]

            cos_t = [blob_sb[:, OFF_COS + n * Q2:OFF_COS + (n + 1) * Q2]
                     for n in range(NT)]
            sin_t = [blob_sb[:, OFF_SIN + n * Q2:OFF_SIN + (n + 1) * Q2]
                     for n in range(NT)]
            rho_t = [blob_sb[:, OFF_RHO + n * Q2:OFF_RHO + (n + 1) * Q2]
                     for n in range(NT)]

            def ucol(b, s):  # contiguous chunk-tap slice [128, Q]
                a = OFF_U + (b * L + s) * Q
                return blob_sb[:, a:a + Q]

            # ---- phase A: G matmuls (per-batch psum, paired sbuf tile) ----
            g_sb = [[None] * 2 for _ in range(NT)]
            for n in range(NT):
                for p in range(2):
                    gs = gpool.tile([128, Q2], f16, tag=f"g{p}{n}")
                    for b in range(BLOCAL):
                        pg = pgpool.tile([128, Q], f32, tag="pg")
                        for s in range(L):
                            nc.tensor.matmul(
                                pg[:], wg(n, p, s), ucol(b, s),
                                start=(s == 0), stop=(s == L - 1),
                            )
                        nc.scalar.copy(gs[:, b * Q:(b + 1) * Q], pg[:])
                    g_sb[n][p] = gs

            # ---- phase B: modulate -> segmented scan -> demodulate ----
            xr_sh = [None] * NT
            xi_sh = [None] * NT
            for n in range(NT):
                gr, gi = g_sb[n][0], g_sb[n][1]
                ct, st = cos_t[n], sin_t[n]
                t1 = tpool.tile([128, Q2], f16, tag="t1")
                t2 = tpool.tile([128, Q2], f16, tag="t2")
                nc.vector.tensor_tensor(t1[:], ct, gr[:], op=op.mult)
                nc.vector.tensor_tensor(t2[:], st, gi[:], op=op.mult)
                ghr = hpool.tile([128, Q2], f16, tag="ghr")
                nc.vector.tensor_tensor(ghr[:], t1[:], t2[:], op=op.add)
                t3 = tpool.tile([128, Q2], f16, tag="t1")
                t4 = tpool.tile([128, Q2], f16, tag="t2")
                nc.vector.tensor_tensor(t3[:], ct, gi[:], op=op.mult)
                nc.vector.tensor_tensor(t4[:], st, gr[:], op=op.mult)
                ghi = hpool.tile([128, Q2], f16, tag="ghi")
                nc.vector.tensor_tensor(ghi[:], t3[:], t4[:], op=op.subtract)
                zr = zpool.tile([128, Q2], f16, tag="zr")
                zi = zpool.tile([128, Q2], f16, tag="zi")
                # rho has col Q zeroed -> state resets at the b=1 boundary
                nc.vector.tensor_tensor_scan(
                    zr[:], rho_t[n], ghr[:], 0.0, op0=op.mult, op1=op.add
                )
                nc.vector.tensor_tensor_scan(
                    zi[:], rho_t[n], ghi[:], 0.0, op0=op.mult, op1=op.add
                )
                # demod into shifted buffers: per batch, col b*(Q+1) = 0
                # (chunk -1), col b*(Q+1)+1+q = X_q; carry reads cols
                # [b*(Q+1), b*(Q+1)+Q).
                xr = xpool.tile([128, 2 * (Q + 1)], f16, tag=f"xr{n}")
                xi = xpool.tile([128, 2 * (Q + 1)], f16, tag=f"xi{n}")
                t5 = tpool.tile([128, Q2], f16, tag="t1")
                t6 = tpool.tile([128, Q2], f16, tag="t2")
                nc.vector.tensor_tensor(t5[:], ct, zr[:], op=op.mult)
                nc.vector.tensor_tensor(t6[:], st, zi[:], op=op.mult)
                t7 = tpool.tile([128, Q2], f16, tag="t7")
                t8 = tpool.tile([128, Q2], f16, tag="t8")
                nc.vector.tensor_tensor(t7[:], st, zr[:], op=op.mult)
                nc.vector.tensor_tensor(t8[:], ct, zi[:], op=op.mult)
                for b in range(BLOCAL):
                    c0 = b * (Q + 1)
                    nc.gpsimd.memset(xr[:, c0:c0 + 1], 0.0)
                    nc.gpsimd.memset(xi[:, c0:c0 + 1], 0.0)
                    nc.vector.tensor_tensor(
                        xr[:, c0 + 1:c0 + 1 + Q],
                        t5[:, b * Q:(b + 1) * Q], t6[:, b * Q:(b + 1) * Q],
                        op=op.subtract,
                    )
                    nc.vector.tensor_tensor(
                        xi[:, c0 + 1:c0 + 1 + Q],
                        t7[:, b * Q:(b + 1) * Q], t8[:, b * Q:(b + 1) * Q],
                        op=op.add,
                    )
                xr_sh[n], xi_sh[n] = xr, xi

            # ---- phase C: in-chunk + carry y matmuls ----
            ysb = ypool.tile([128, UCOLS], f16, tag="y")
            for j in range(L):
                for b in range(BLOCAL):
                    py = pypool.tile([128, Q], f32, tag="py")
                    for sp in range(j + 1):
                        nc.tensor.matmul(
                            py[:], pw(j - sp), ucol(b, sp),
                            start=(sp == 0), stop=False,
                        )
                    for p in range(2):
                        xs = xr_sh if p == 0 else xi_sh
                        for n in range(NT):
                            last = (p == 1 and n == NT - 1)
                            c0 = b * (Q + 1)
                            nc.tensor.matmul(
                                py[:], cw(j, p, n), xs[n][:, c0:c0 + Q],
                                start=False, stop=last,
                            )
                    nc.scalar.copy(
                        ysb[:, (b * L + j) * Q:(b * L + j + 1) * Q], py[:]
                    )
            for b in range(BLOCAL):
                nc.gpsimd.dma_start(
                    yout[:, b * T:(b + 1) * T],
                    ysb[:, b * T:(b + 1) * T],
                )

    _legalize_multi_waits(nc)
    return nc


def _legalize_multi_waits(nc):
    """This walrus build accepts a single sync wait per instruction; split
    any multi-wait instruction into same-engine single-wait NoOps + the
    original carrying the last wait (program order chains them)."""
    import bass_rust
    from concourse import mybir

    uid = [0]
    for fn in nc.m.functions:
        for bb in fn.blocks:
            insts = bb.instructions
            new = []
            changed = False
            for inst in insts:
                si = inst.sync_info
                if si is not None and len(si.on_wait) > 1:
                    waits = list(si.on_wait)
                    for w in waits[:-1]:
                        uid[0] += 1
                        new.append(mybir.InstNoOp(
                            name=f"mwsplit-{uid[0]}",
                            engine=inst.engine,
                            ins=[], outs=[],
                            sync_info=bass_rust.SyncInfo(on_wait=[w], on_update=[]),
                        ))
                    inst.sync_info = bass_rust.SyncInfo(
                        on_wait=[waits[-1]], on_update=list(si.on_update)
                    )
                    changed = True
                new.append(inst)
            if changed:
                bb.instructions = new


def _host_prep(A_re, A_im, B_re, B_im, C_re, C_im, D_w):
    """fp64 eigendecomposition + fp16 weight/table layouts (shared blob
    columns, everything except the per-core u block)."""
    A = A_re.astype(np.float64) + 1j * A_im.astype(np.float64)
    w, V = np.linalg.eig(A)
    Vinv = np.linalg.inv(V)
    Bt = Vinv @ (B_re.astype(np.float64) + 1j * B_im.astype(np.float64))  # [N, IN]
    Ct = (C_re.astype(np.float64) + 1j * C_im.astype(np.float64)) @ V     # [OUT, N]

    wp = w[None, :] ** np.arange(L + 1)[:, None]  # wp[k] = w^k

    parts = []
    # WG tiles: lhsT [128 in, 128 modes] for (n, p, s)
    Ms = [wp[L - 1 - s][:, None] * Bt for s in range(L)]  # [N, IN]
    for n in range(NT):
        for p in range(2):
            for s in range(L):
                comp = Ms[s].real if p == 0 else Ms[s].imag
                parts.append(comp[n * 128:(n + 1) * 128, :].T)
    # PW tiles: lhsT [128 in, 128 out]
    Pk = [np.real(Ct @ (wp[k][:, None] * Bt)) for k in range(L)]
    Pk[0] = Pk[0] + D_w.astype(np.float64)
    for k in range(L):
        parts.append(Pk[k].T)
    # tables, cols (b, q); rho zeroed at the batch boundary col Q
    wt = w ** L
    rho = np.abs(wt)
    phi = np.angle(wt)
    qs = np.arange(Q, dtype=np.float64)
    cos1 = np.cos(phi[:, None] * qs[None, :])
    sin1 = np.sin(phi[:, None] * qs[None, :])
    rho1 = np.broadcast_to(rho[:, None], (N, Q)).copy()
    rho2 = np.concatenate([rho1, rho1], axis=1)
    rho2[:, Q] = 0.0
    for tab in (np.concatenate([cos1, cos1], 1), np.concatenate([sin1, sin1], 1),
                rho2):
        parts.append(tab.reshape(NT, 128, Q2).transpose(1, 0, 2).reshape(128, NT * Q2))
    # CW tiles: lhsT [128 modes, 128 out] for (j, p, n); p=0 Re, p=1 -Im
    for j in range(L):
        Cj = Ct * wp[j + 1][None, :]  # [OUT, N]
        for comp in (Cj.real, -Cj.imag):
            for n in range(NT):
                parts.append(comp[:, n * 128:(n + 1) * 128].T)
    out = np.concatenate([np.ascontiguousarray(p) for p in parts], axis=1)
    assert out.shape == (128, BLOBW - UCOLS)
    return out.astype(__import__("ml_dtypes").bfloat16)


def _ensure_axon_hooks():
    """Provide antenv.axon_hooks if the image lacks it (needed only for
    trace=True NTFF profiling; run path works without)."""
    import types
    try:
        from antenv import axon_hooks  # noqa: F401
        return
    except ImportError:
        pass
    try:
        import antenv
        mod = types.ModuleType("antenv.axon_hooks")
        _hook = [None]
        mod.set_axon_ntff_profile_hook = lambda h: _hook.__setitem__(0, h)
        mod.get_axon_ntff_profile_hook = lambda: _hook[0]
        sys.modules["antenv.axon_hooks"] = mod
        antenv.axon_hooks = mod
        if "/root/.axon_site" not in sys.path:
            sys.path.insert(0, "/root/.axon_site")
        from trn_agent_boot.trn_boot import _ntff_profile_via_ctypes
        h = _ntff_profile_via_ctypes("/opt/axon/libaxon_pjrt.so")
        if h is not None:
            mod.set_axon_ntff_profile_hook(h)
    except Exception:
        pass


def kernel(u, A_re, A_im, B_re, B_im, C_re, C_im, D_w, output_bias):
    global LAST_RESULT, _NC_CACHE
    from concourse import bass_utils

    _ensure_axon_hooks()

    u = np.asarray(u, dtype=np.float32)
    shared = _host_prep(
        np.asarray(A_re), np.asarray(A_im), np.asarray(B_re), np.asarray(B_im),
        np.asarray(C_re), np.asarray(C_im), np.asarray(D_w)
    )

    if _NC_CACHE is None:
        _NC_CACHE = _build_nc()
    nc = _NC_CACHE

    in_maps = []
    for k in range(NCORES):
        u_pair = u[BLOCAL * k:BLOCAL * (k + 1)]  # [2, T, IN]
        # chunk-tap-major: col (b, s, q) = u[b, q*L+s, i]
        ut = np.ascontiguousarray(
            u_pair.reshape(BLOCAL, Q, L, IN).transpose(3, 0, 2, 1)
            .reshape(128, UCOLS)
        ).astype(__import__("ml_dtypes").bfloat16)
        in_maps.append({"blob": np.concatenate([ut, shared], axis=1)})

    res = bass_utils.run_bass_kernel_spmd(nc, in_maps, core_ids=list(range(NCORES)))
    LAST_RESULT = res

    y = np.empty((BATCH, T, OUT), dtype=np.float32)
    for k in range(NCORES):
        yd = res.results[k]["y"].astype(np.float32)  # [OUT, (b, j, q)]
        # y[b, q*L+j, o] = yd[o, (b*L+j)*Q+q]
        y[BLOCAL * k:BLOCAL * (k + 1)] = (
            yd.reshape(OUT, BLOCAL, L, Q).transpose(1, 3, 2, 0)
            .reshape(BLOCAL, T, OUT)
        )
    y += np.asarray(output_bias, dtype=np.float32)
    return y
